# revision 12
# baseline (speedup 1.0000x reference)
"""Trainium2 Bass kernel for nn_EncoderProcesserDecoder (GNN message passing).

8 NeuronCores SPMD. Nodes partitioned by core; edges partitioned by receiver
core, sorted by (receiver 128-node block, sender table half), padded to
128-edge chunks with a schedule shared across cores (one NEFF). Per block:
sender/receiver MLPs on local nodes; the sender table (+ edge-L4 bias folded)
is AllGathered (bf16) and rows are fetched with batched dma_gather (int16
indices; lo/hi table halves). Scatter = one-hot selection matmuls into
per-node-block PSUM; per-edge receiver term replaced by exact deg[n]*r[n] at
agg init. Edge MLP runs in transposed-activation layout, f32 storage, fp32r
matmuls; edge residual stream stored f32 transposed in HBM.
"""
import numpy as np
import ml_dtypes

N = 50000
E = 800000
H = 128
NC = 8
NPC = N // NC
NBLK = 49
NPAD = NBLK * 128
TROWS = NC * NPAD
THALF = TROWS // 2
CHUNK = 128
TILE = 512
CPT = TILE // CHUNK
GCALL = 16

_BUILD_CACHE = {}


# ---------------------------------------------------------------- host prep
def table_row(n):
    return (n // NPC) * NPAD + (n % NPC)


def prep_edges(edge_index):
    snd = np.asarray(edge_index[0], dtype=np.int64)
    rcv = np.asarray(edge_index[1], dtype=np.int64)
    core_of = rcv // NPC
    srow = table_row(snd)
    shalf = (srow >= THALF).astype(np.int64)

    per_core = []
    cnt = np.zeros((NC, NBLK, 2), np.int64)
    for c in range(NC):
        m = core_of == c
        sr_c, sh_c = srow[m], shalf[m]
        rloc = rcv[m] - c * NPC
        blk = rloc // 128
        order = np.lexsort((rloc, sh_c, blk))
        per_core.append((sr_c[order], rloc[order], sh_c[order], blk[order],
                         np.where(m)[0][order]))
        cnt[c] = np.bincount(blk * 2 + sh_c,
                             minlength=NBLK * 2).reshape(NBLK, 2)

    chunks = np.zeros((NBLK, 2), np.int64)
    for b in range(NBLK):
        for h in (0, 1):
            chunks[b, h] = int(np.ceil(cnt[:, b, h].max() / CHUNK))
    total_chunks = int(chunks.sum())
    chunks[NBLK - 1, 1] += (-total_chunks) % CPT
    total_chunks = int(chunks.sum())
    EPAD = total_chunks * CHUNK

    chunk_blk = np.zeros(total_chunks, np.int64)
    chunk_half = np.zeros(total_chunks, np.int64)
    j = 0
    for h in (0, 1):
        for b in range(NBLK):
            for _ in range(int(chunks[b, h])):
                chunk_blk[j] = b
                chunk_half[j] = h
                j += 1
    n_lo_chunks = int(chunks[:, 0].sum())

    cores = []
    for c in range(NC):
        sr_c, rloc_c, sh_c, blk_c, orig_c = per_core[c]
        srow_s = np.zeros(EPAD, np.int64)
        rloc_s = np.full(EPAD, -1, np.int64)
        eidx_s = np.full(EPAD, -1, np.int64)
        pos = 0
        for h in (0, 1):
            for b in range(NBLK):
                m = (blk_c == b) & (sh_c == h)
                k = int(m.sum())
                cap = int(chunks[b, h]) * CHUNK
                srow_s[pos:pos + k] = sr_c[m]
                rloc_s[pos:pos + k] = rloc_c[m]
                eidx_s[pos:pos + k] = orig_c[m]
                pos += cap
        cores.append(dict(srow=srow_s, rloc=rloc_s, eidx=eidx_s))

    sched = dict(chunks=chunks, chunk_blk=chunk_blk, chunk_half=chunk_half,
                 n_lo_chunks=n_lo_chunks, EPAD=EPAD, total_chunks=total_chunks)
    return cores, sched


def wrap_idx16(flat):
    n = len(flat)
    block = flat.reshape(n // 16, 16).T.astype(np.int16)
    return np.tile(block, (8, 1)).copy()


def extract_params(params):
    def ml(layers):
        return [(np.asarray(W, np.float32), np.asarray(b, np.float32))
                for W, b in layers]
    out = dict(enc_node=ml(params['enc_node']), enc_edge=ml(params['enc_edge']),
               dec=ml(params['dec']), blocks=[])
    for blk in params['blocks']:
        out['blocks'].append({k: ml(blk[k]) for k in
                              ('sender', 'receiver', 'edge', 'node')})
    return out


def weight_names():
    WN = []
    for i in range(1, 4):
        WN.append(("enc_node", 0, i))
    for i in range(1, 4):
        WN.append(("enc_edge", 0, i))
    for k in range(2):
        for part in ("sender", "receiver", "edge"):
            for i in range(4):
                WN.append((part, k, i))
        WN.append(("nodeA", k, 0))
        WN.append(("nodeB", k, 0))
        for i in range(1, 4):
            WN.append(("node", k, i))
        WN.append(("sfold", k, 0))   # bias-only: b4_sender + b4_edge
    for i in range(4):
        WN.append(("dec", 0, i))
    return WN


WNAMES = weight_names()
WIDX = {w: i for i, w in enumerate(WNAMES)}
NW = len(WNAMES)


def pack_weights(P):
    Wp = np.zeros((128, NW * 128), np.float32)  # cast to bf16 at the end
    Bp = np.zeros((128, NW), np.float32)

    def put(key, Wm, bv):
        i = WIDX[key]
        if Wm is not None:
            Wp[:Wm.shape[0], i * 128:i * 128 + Wm.shape[1]] = Wm
        if bv is not None:
            Bp[:len(bv), i] = bv

    for i in range(1, 4):
        put(("enc_node", 0, i), P['enc_node'][i][0], P['enc_node'][i][1])
        put(("enc_edge", 0, i), P['enc_edge'][i][0], P['enc_edge'][i][1])
    for k in range(2):
        blk = P['blocks'][k]
        for part in ("sender", "receiver", "edge"):
            for i in range(4):
                put((part, k, i), blk[part][i][0], blk[part][i][1])
        put(("nodeA", k, 0), blk['node'][0][0][:H], blk['node'][0][1])
        put(("nodeB", k, 0), blk['node'][0][0][H:], None)
        for i in range(1, 4):
            put(("node", k, i), blk['node'][i][0], blk['node'][i][1])
        put(("sfold", k, 0), None,
            blk['sender'][3][1] + blk['edge'][3][1])
    for i in range(4):
        put(("dec", 0, i), P['dec'][i][0], P['dec'][i][1])
    return Wp.astype(ml_dtypes.bfloat16), Bp


# ---------------------------------------------------------------- bass build
def build_kernel(sched):
    import concourse.bass as bass
    from concourse import bacc
    import concourse.mybir as mybir
    import concourse.tile as tile

    dt = mybir.dt
    F32, BF16, F32R, I16 = dt.float32, dt.bfloat16, dt.float32r, dt.int16
    AF = mybir.ActivationFunctionType
    ALU = mybir.AluOpType

    EPAD = sched['EPAD']
    NCHUNK = sched['total_chunks']
    NLO = sched['n_lo_chunks']
    T = EPAD // TILE
    chunk_blk = sched['chunk_blk']
    chunk_half = sched['chunk_half']

    run_id = [(int(chunk_half[j]), int(chunk_blk[j])) for j in range(NCHUNK)]
    run_start = [j == 0 or run_id[j] != run_id[j - 1] for j in range(NCHUNK)]
    run_end = [j == NCHUNK - 1 or run_id[j] != run_id[j + 1]
               for j in range(NCHUNK)]

    def call_ranges(lo, hi):
        out = []
        a = lo
        while a < hi:
            out.append((a, min(a + GCALL, hi)))
            a = min(a + GCALL, hi)
        return out
    s_calls = call_ranges(0, NLO) + call_ranges(NLO, NCHUNK)
    r_calls = call_ranges(0, NCHUNK)
    s_call_of_chunk = {}
    for ci, (a, b) in enumerate(s_calls):
        for j in range(a, b):
            s_call_of_chunk[j] = ci
    r_call_of_chunk = {}
    for ci, (a, b) in enumerate(r_calls):
        for j in range(a, b):
            r_call_of_chunk[j] = ci

    nc = bacc.Bacc(None)

    ea_in = nc.dram_tensor("ea", [4, EPAD], BF16, kind="ExternalInput")
    x_in = nc.dram_tensor("x", [4, NPAD], BF16, kind="ExternalInput")
    w_in = nc.dram_tensor("w", [128, NW * 128], BF16, kind="ExternalInput")
    waug_in = nc.dram_tensor("waug", [4, 2 * 128], BF16, kind="ExternalInput")
    w4m2bf_in = nc.dram_tensor("w4m2bf", [128, 128], BF16, kind="ExternalInput")
    b_in = nc.dram_tensor("b", [128, NW], F32, kind="ExternalInput")
    iota_in = nc.dram_tensor("iota", [128, 128], F32, kind="ExternalInput")
    ibf_in = nc.dram_tensor("ibf", [128, 128], BF16, kind="ExternalInput")
    if32_in = nc.dram_tensor("if32", [128, 128], F32, kind="ExternalInput")
    sidx_in = nc.dram_tensor("sidx", [128, NCHUNK * 8], I16, kind="ExternalInput")
    ridx_in = nc.dram_tensor("ridx", [128, NCHUNK * 8], I16, kind="ExternalInput")
    rcvc_in = nc.dram_tensor("rcvc", [128, NCHUNK], F32, kind="ExternalInput")
    deg_in = nc.dram_tensor("deg", [128, NBLK], F32, kind="ExternalInput")
    y_out = nc.dram_tensor("y", [NPAD, 2], F32, kind="ExternalOutput")
    dbg_nh0 = nc.dram_tensor("dbg_nh0", [128, NPAD], BF16, kind="ExternalOutput")
    dbg_agg1 = nc.dram_tensor("dbg_agg1", [128, NBLK * 128], F32, kind="ExternalOutput")
    dbg_nh1 = nc.dram_tensor("dbg_nh1", [128, NPAD], BF16, kind="ExternalOutput")
    dbg_stab = nc.dram_tensor("dbg_stab", [256, H], BF16, kind="ExternalOutput")
    dbg_eh = nc.dram_tensor("dbg_eh", [128, TILE], BF16, kind="ExternalOutput")
    dbg_agg2 = nc.dram_tensor("dbg_agg2", [128, NBLK * 128], F32, kind="ExternalOutput")
    dbg_nh2 = nc.dram_tensor("dbg_nh2", [128, NPAD], BF16, kind="ExternalOutput")
    dbg_stab1 = nc.dram_tensor("dbg_stab1", [256, H], BF16, kind="ExternalOutput")
    dbg_x2r = nc.dram_tensor("dbg_x2r", [128, TILE], BF16, kind="ExternalOutput")
    dbg_ysb = nc.dram_tensor("dbg_ysb", [128, NBLK * 2], F32, kind="ExternalOutput")

    ehT = nc.dram_tensor("ehT", [128, EPAD], BF16)
    rtab0 = nc.dram_tensor("rtab0", [NPAD, H], BF16)

    RG = [list(range(NC))]
    col_tiles = [(i * TILE, min(NPAD, (i + 1) * TILE))
                 for i in range((NPAD + TILE - 1) // TILE)]

    with tile.TileContext(nc) as tc:
        with (
            tc.tile_pool(name="dram", bufs=1, space="DRAM") as dram,
            tc.tile_pool(name="persist", bufs=1) as pp,
            tc.tile_pool(name="nodebuf", bufs=2) as npool,
            tc.tile_pool(name="work", bufs=2) as wp,
            tc.tile_pool(name="gath", bufs=3) as gp,
            tc.tile_pool(name="small", bufs=2) as sp,
            tc.tile_pool(name="psA", bufs=2, space="PSUM") as psA,
            tc.tile_pool(name="psT", bufs=2, space="PSUM") as psT,
            tc.tile_pool(name="psN", bufs=2, space="PSUM") as psN,
            tc.tile_pool(name="psG", bufs=2, space="PSUM") as psG,
        ):
            W = pp.tile([128, NW * 128], BF16)
            nc.sync.dma_start(W[:], w_in[:])
            Waug = pp.tile([4, 2 * 128], BF16)
            nc.sync.dma_start(Waug[:], waug_in[:])
            W4m2bf = pp.tile([128, 128], BF16)
            nc.sync.dma_start(W4m2bf[:], w4m2bf_in[:])
            B = pp.tile([128, NW], F32)
            nc.sync.dma_start(B[:], b_in[:])
            IOTA = pp.tile([128, 128], F32)
            nc.sync.dma_start(IOTA[:], iota_in[:])
            IBF = pp.tile([128, 128], BF16)
            nc.sync.dma_start(IBF[:], ibf_in[:])
            IF32 = pp.tile([128, 128], F32)
            nc.sync.dma_start(IF32[:], if32_in[:])
            RCVC = pp.tile([128, NCHUNK], F32)
            nc.sync.dma_start(RCVC[:], rcvc_in[:])
            DEG = pp.tile([128, NBLK], F32)
            nc.sync.dma_start(DEG[:], deg_in[:])
            AGG = pp.tile([128, NBLK * 128], F32, tag="agg")

            def w_ap(key):
                i = WIDX[key]
                return W[:, i * 128:(i + 1) * 128]

            def b_ap(key):
                i = WIDX[key]
                return B[:, i:i + 1]

            def wr(ap):
                return ap

            def chainT(x_ap, keys, n_free, relu_engines, out_tag,
                       out_dtype=None):
                cur = x_ap
                nkeys = len(keys)
                for li, key in enumerate(keys):
                    ps = psA.tile([128, TILE], F32, tag="mlp")
                    if isinstance(key, tuple) and key[0] == "aug":
                        lhs = Waug[:, key[1] * 128:(key[1] + 1) * 128]
                        nc.tensor.matmul(ps[:, :n_free], wr(lhs), wr(cur),
                                         start=True, stop=True)
                        bkey = None
                    elif isinstance(key, tuple) and key[0] == "dual":
                        nc.tensor.matmul(ps[:, :n_free], wr(w_ap(key[1])),
                                         wr(cur), start=True, stop=False)
                        nc.tensor.matmul(ps[:, :n_free], wr(w_ap(key[3])),
                                         wr(key[2]), start=False, stop=True)
                        bkey = key[1]
                    else:
                        nc.tensor.matmul(ps[:, :n_free], wr(w_ap(key)),
                                         wr(cur), start=True, stop=True)
                        bkey = key
                    if li == nkeys - 1:
                        o = wp.tile([128, TILE], out_dtype or F32, tag=out_tag)
                        nc.vector.tensor_scalar(
                            out=o[:, :n_free], in0=ps[:, :n_free],
                            scalar1=b_ap(bkey) if bkey else 0.0,
                            scalar2=None, op0=ALU.add)
                        return o
                    o = wp.tile([128, TILE], BF16, tag=f"z{li % 3}")
                    eng = relu_engines[li % len(relu_engines)]
                    if eng == "act":
                        nc.scalar.activation(
                            o[:, :n_free], ps[:, :n_free], AF.Relu,
                            bias=b_ap(bkey) if bkey else 0.0)
                    else:
                        nc.vector.tensor_scalar(
                            out=o[:, :n_free], in0=ps[:, :n_free],
                            scalar1=b_ap(bkey) if bkey else 0.0,
                            scalar2=0.0, op0=ALU.add, op1=ALU.max)
                    cur = o[:, :n_free]

            # ================= P1: node encoder ===========================
            node_h0 = npool.tile([128, NPAD], BF16, tag="node_h")
            for (a, b) in col_tiles:
                xa_t = sp.tile([4, TILE], BF16, tag="xa")
                nc.sync.dma_start(xa_t[:, :b - a], x_in[:, a:b])
                o = chainT(xa_t[:, :b - a],
                           [("aug", 1), ("enc_node", 0, 1),
                            ("enc_node", 0, 2), ("enc_node", 0, 3)],
                           b - a, ["act", "dve", "act"], "co",
                           out_dtype=BF16)
                nc.vector.tensor_copy(node_h0[:, a:b], o[:, :b - a])

            def node_tables(k, node_h_t, cc_in, cc_out, rtab_t, init_deg_r):
                for (a, b) in col_tiles:
                    x3 = chainT(node_h_t[:, a:b],
                                [("sender", k, 0), ("sender", k, 1),
                                 ("sender", k, 2)],
                                b - a, ["act", "dve"], "co")
                    x3r = wp.tile([128, TILE], BF16, tag="ro")
                    nc.vector.tensor_scalar(out=x3r[:, :b - a],
                                            in0=x3[:, :b - a], scalar1=0.0,
                                            scalar2=None, op0=ALU.max)
                    nch = (b - a) // 128
                    sflip = wp.tile([128, TILE], BF16, tag="sfl")
                    for cix in range(nch):
                        ps = psN.tile([128, TILE], F32, tag="pn")
                        nc.tensor.matmul(
                            ps[:, :128],
                            wr(x3r[:, cix * 128:(cix + 1) * 128]),
                            wr(w_ap(("sender", k, 3))), start=True, stop=True)
                        nc.scalar.activation(
                            sflip[:, cix * 128:(cix + 1) * 128], ps[:, :128],
                            AF.Identity, bias=b_ap(("sfold", k, 0)))
                    for cix in range(nch):
                        ra = a + cix * 128
                        nc.sync.dma_start(
                            cc_in[ra:ra + 128, :],
                            sflip[:, cix * 128:(cix + 1) * 128])
                nc.gpsimd.collective_compute(
                    "AllGather", ALU.bypass, replica_groups=RG,
                    ins=[cc_in.opt()], outs=[cc_out.opt()])

                for (a, b) in col_tiles:
                    x3 = chainT(node_h_t[:, a:b],
                                [("receiver", k, 0), ("receiver", k, 1),
                                 ("receiver", k, 2)],
                                b - a, ["dve", "act"], "co")
                    x3r = wp.tile([128, TILE], BF16, tag="ro")
                    nc.vector.tensor_scalar(out=x3r[:, :b - a],
                                            in0=x3[:, :b - a], scalar1=0.0,
                                            scalar2=None, op0=ALU.max)
                    nch = (b - a) // 128
                    for cix in range(nch):
                        blk_i = a // 128 + cix
                        ps = psN.tile([128, TILE], F32, tag="pn")
                        nc.tensor.matmul(
                            ps[:, :128],
                            wr(x3r[:, cix * 128:(cix + 1) * 128]),
                            wr(w_ap(("receiver", k, 3))), start=True, stop=True)
                        if init_deg_r:
                            nc.vector.tensor_scalar(
                                out=AGG[:, blk_i * 128:(blk_i + 1) * 128],
                                in0=ps[:, :128],
                                scalar1=b_ap(("receiver", k, 3)),
                                scalar2=DEG[:, blk_i:blk_i + 1],
                                op0=ALU.add, op1=ALU.mult)
                        if rtab_t is not None:
                            rbf = sp.tile([128, 128], BF16, tag="rbf")
                            nc.scalar.activation(
                                rbf[:], ps[:, :128], AF.Identity,
                                bias=b_ap(("receiver", k, 3)))
                            nc.sync.dma_start(
                                rtab_t[blk_i * 128:(blk_i + 1) * 128, :],
                                rbf[:])

            nc.sync.dma_start(dbg_nh0[:], node_h0[:])
            cc_in0 = dram.tile([NPAD, H], BF16, tag="cci0")
            cc_out0 = dram.tile([TROWS, H], BF16, tag="cco0")
            nc.vector.memset(AGG[:], 0.0)
            node_tables(0, node_h0, cc_in0, cc_out0, rtab0, False)
            nc.gpsimd.dma_start(dbg_stab[:], cc_out0[0:256, :])

            # ================= edge sweep =================================
            def edge_sweep(k, cc_out, with_residual):
                s_tiles = {}
                r_tiles = {}

                def issue_s(ci):
                    a, b = s_calls[ci]
                    nidx = (b - a) * CHUNK
                    gt = gp.tile([128, GCALL * 128], BF16, tag="sg")
                    it = sp.tile([128, GCALL * 8], I16, tag="sidx")
                    nc.sync.dma_start(it[:, :nidx // 16],
                                      sidx_in[:, a * 8:a * 8 + nidx // 16])
                    src = cc_out[:THALF, :] if a < NLO else cc_out[THALF:, :]
                    nc.gpsimd.dma_gather(
                        out_ap=gt[:, :nidx // 128 * 128].rearrange(
                            "p (c h) -> p c h", h=128),
                        in_ap=src, idxs_ap=it[:, :nidx // 16],
                        num_idxs=nidx, num_idxs_reg=nidx, elem_size=H,
                        single_packet=False)
                    s_tiles[ci] = (gt, a)

                def issue_r(ci):
                    a, b = r_calls[ci]
                    nidx = (b - a) * CHUNK
                    gt = gp.tile([128, GCALL * 128], BF16, tag="rg")
                    it = sp.tile([128, GCALL * 8], I16, tag="ridx")
                    nc.sync.dma_start(it[:, :nidx // 16],
                                      ridx_in[:, a * 8:a * 8 + nidx // 16])
                    nc.gpsimd.dma_gather(
                        out_ap=gt[:, :nidx // 128 * 128].rearrange(
                            "p (c h) -> p c h", h=128),
                        in_ap=rtab0[:], idxs_ap=it[:, :nidx // 16],
                        num_idxs=nidx, num_idxs_reg=nidx, elem_size=H,
                        single_packet=False)
                    r_tiles[ci] = (gt, a)

                agg_ps = {}
                for t in range(T):
                    esl = slice(t * TILE, (t + 1) * TILE)
                    c0 = t * CPT
                    for j in range(c0, c0 + CPT):
                        ci = s_call_of_chunk[j]
                        if ci not in s_tiles:
                            issue_s(ci)
                        if with_residual:
                            ri = r_call_of_chunk[j]
                            if ri not in r_tiles:
                                issue_r(ri)

                    if k == 0:
                        ea_t = sp.tile([4, TILE], BF16, tag="ea")
                        nc.sync.dma_start(ea_t[:], ea_in[:, esl])
                        h0 = chainT(ea_t[:, :],
                                    [("aug", 0), ("enc_edge", 0, 1),
                                     ("enc_edge", 0, 2), ("enc_edge", 0, 3)],
                                    TILE, ["dve", "act", "dve"], "h0",
                                    out_dtype=BF16)
                    else:
                        h0 = wp.tile([128, TILE], BF16, tag="h0")
                        nc.sync.dma_start(h0[:], ehT[:, esl])

                    cur = h0[:, :]
                    for li in range(3):
                        ps = psA.tile([128, TILE], F32, tag="mlp")
                        nc.tensor.matmul(ps[:], wr(w_ap(("edge", k, li))),
                                         wr(cur), start=True, stop=True)
                        o = wp.tile([128, TILE], BF16, tag=f"z{li}")
                        if li % 2 == 0:
                            nc.scalar.activation(o[:], ps[:], AF.Relu,
                                                 bias=b_ap(("edge", k, li)))
                        else:
                            nc.vector.tensor_scalar(
                                out=o[:], in0=ps[:],
                                scalar1=b_ap(("edge", k, li)),
                                scalar2=0.0, op0=ALU.add, op1=ALU.max)
                        cur = o[:, :]

                    if with_residual:
                        pT = psT.tile([128, TILE], F32, tag="pt")
                        nc.tensor.matmul(pT[:], wr(w_ap(("edge", k, 3))),
                                         wr(cur), start=True, stop=False)
                        for cix in range(CPT):
                            j = c0 + cix
                            gt, ga = s_tiles[s_call_of_chunk[j]]
                            off = (j - ga) * 128
                            nc.tensor.matmul(
                                pT[:, cix * 128:(cix + 1) * 128],
                                gt[:, off:off + 128], IBF[:],
                                start=False, stop=False, skip_group_check=True)
                        for cix in range(CPT):
                            j = c0 + cix
                            gt, ga = r_tiles[r_call_of_chunk[j]]
                            off = (j - ga) * 128
                            nc.tensor.matmul(
                                pT[:, cix * 128:(cix + 1) * 128],
                                gt[:, off:off + 128], IBF[:],
                                start=False, stop=(cix == CPT - 1),
                                skip_group_check=True)
                        ebf = wp.tile([128, TILE], BF16, tag="ebf")
                        nc.scalar.activation(ebf[:], pT[:], AF.Copy)
                        eh1 = wp.tile([128, TILE], BF16, tag="eh1")
                        nc.vector.tensor_tensor(out=eh1[:], in0=pT[:],
                                                in1=h0[:, :], op=ALU.add)
                        nc.sync.dma_start(ehT[:, esl], eh1[:])
                        pN = psN.tile([128, TILE], BF16, tag="pn")
                        for cix in range(CPT):
                            nc.tensor.transpose(
                                pN[:, cix * 128:(cix + 1) * 128].bitcast(BF16),
                                ebf[:, cix * 128:(cix + 1) * 128], IBF[:])
                        en = wp.tile([128, TILE], BF16, tag="en")
                        nc.scalar.activation(en[:], pN[:], AF.Copy)
                    else:
                        x3bf = cur
                        pN = psN.tile([128, TILE], F32, tag="pn")
                        for cix in range(CPT):
                            j = c0 + cix
                            nc.tensor.matmul(
                                pN[:, cix * 128:(cix + 1) * 128],
                                x3bf[:, cix * 128:(cix + 1) * 128],
                                W4m2bf[:], start=True, stop=False,
                                skip_group_check=True)
                            gt, ga = s_tiles[s_call_of_chunk[j]]
                            off = (j - ga) * 128
                            nc.tensor.matmul(
                                pN[:, cix * 128:(cix + 1) * 128],
                                IBF[:], gt[:, off:off + 128],
                                start=False, stop=True, skip_group_check=True)
                        en = wp.tile([128, TILE], BF16, tag="en")
                        nc.scalar.activation(en[:], pN[:], AF.Copy)

                    oh = wp.tile([128, TILE], BF16, tag="oh")
                    for cix in range(CPT):
                        j = c0 + cix
                        nc.vector.tensor_scalar(
                            out=oh[:, cix * 128:(cix + 1) * 128],
                            in0=IOTA[:], scalar1=RCVC[:, j:j + 1],
                            scalar2=None, op0=ALU.is_equal)
                    for cix in range(CPT):
                        j = c0 + cix
                        rid = run_id[j]
                        if run_start[j]:
                            pagg_t = psG.tile([128, 128], F32, tag="pagg")
                            agg_ps[rid] = pagg_t
                        pg = agg_ps[rid]
                        nc.tensor.matmul(
                            pg[:], oh[:, cix * 128:(cix + 1) * 128],
                            en[:, cix * 128:(cix + 1) * 128],
                            start=run_start[j], stop=run_end[j],
                            skip_group_check=True)
                        if run_end[j]:
                            bb = rid[1]
                            nc.vector.tensor_tensor(
                                out=AGG[:, bb * 128:(bb + 1) * 128],
                                in0=AGG[:, bb * 128:(bb + 1) * 128],
                                in1=pg[:], op=ALU.add)
                            del agg_ps[rid]

            edge_sweep(0, cc_out0, True)
            nc.sync.dma_start(dbg_agg1[:], AGG[:])
            dbg_eh_sb = sp.tile([128, TILE], BF16, tag="dbgeh")
            nc.sync.dma_start(dbg_eh_sb[:], ehT[:, 0:TILE])
            nc.sync.dma_start(dbg_eh[:], dbg_eh_sb[:])

            def node_update(k, node_h_t):
                new_h = npool.tile([128, NPAD], BF16, tag="node_h")
                for (a, b) in col_tiles:
                    nch = (b - a) // 128
                    aggT = wp.tile([128, TILE], BF16, tag="aggT")
                    for cix in range(nch):
                        blk_i = a // 128 + cix
                        ps = psN.tile([128, TILE], F32, tag="pn")
                        nc.tensor.transpose(
                            ps[:, :128],
                            AGG[:, blk_i * 128:(blk_i + 1) * 128], IF32[:])
                        nc.scalar.activation(
                            aggT[:, cix * 128:(cix + 1) * 128], ps[:, :128],
                            AF.Copy)
                    o = chainT(node_h_t[:, a:b],
                               [("dual", ("nodeA", k, 0), aggT[:, :b - a],
                                 ("nodeB", k, 0)),
                                ("node", k, 1), ("node", k, 2),
                                ("node", k, 3)],
                               b - a, ["act", "dve", "act"], "co",
                               out_dtype=F32)
                    nc.vector.tensor_tensor(out=new_h[:, a:b],
                                            in0=o[:, :b - a],
                                            in1=node_h_t[:, a:b],
                                            op=ALU.add)
                return new_h

            node_h1 = node_update(0, node_h0)
            nc.sync.dma_start(dbg_nh1[:], node_h1[:])

            cc_in1 = dram.tile([NPAD, H], BF16, tag="cci1")
            cc_out1 = dram.tile([TROWS, H], BF16, tag="cco1")
            node_tables(1, node_h1, cc_in1, cc_out1, None, True)
            nc.gpsimd.dma_start(dbg_stab1[:], cc_out1[0:256, :])

            edge_sweep(1, cc_out1, False)
            nc.sync.dma_start(dbg_agg2[:], AGG[:])

            node_h2 = node_update(1, node_h1)
            nc.sync.dma_start(dbg_nh2[:], node_h2[:])

            ysb = pp.tile([128, NBLK * 2], F32, tag="ysb")
            for (a, b) in col_tiles:
                x2 = chainT(node_h2[:, a:b],
                            [("dec", 0, 0), ("dec", 0, 1), ("dec", 0, 2)],
                            b - a, ["act", "dve"], "co")
                x2r = wp.tile([128, TILE], BF16, tag="ro")
                nc.vector.tensor_scalar(out=x2r[:, :b - a], in0=x2[:, :b - a],
                                        scalar1=0.0, scalar2=None, op0=ALU.max)
                if a == 0:
                    nc.sync.dma_start(dbg_x2r[:], x2r[:, :TILE])
                nch = (b - a) // 128
                for cix in range(nch):
                    blk_i = a // 128 + cix
                    ps = psN.tile([128, TILE], F32, tag="pn")
                    nc.tensor.matmul(
                        ps[:, :2], x2r[:, cix * 128:(cix + 1) * 128],
                        w_ap(("dec", 0, 3))[:, :2], start=True, stop=True)
                    nc.vector.tensor_copy(
                        out=ysb[:, blk_i * 2:(blk_i + 1) * 2], in_=ps[:, :2])
            nc.sync.dma_start(dbg_ysb[:], ysb[:])
            for bb in range(NBLK):
                nc.sync.dma_start(y_out[bb * 128:(bb + 1) * 128, :],
                                  ysb[:, bb * 2:(bb + 1) * 2])

    nc.compile()
    return nc


# ---------------------------------------------------------------- runner
def kernel(**inputs):
    from concourse.bass_utils import run_bass_kernel_spmd

    x = np.asarray(inputs['x'], np.float32)
    edge_attr = np.asarray(inputs['edge_attr'], np.float32)
    edge_index = np.asarray(inputs['edge_index'])
    P = extract_params(inputs['params'])

    cores, sched = prep_edges(edge_index)
    key = (sched['EPAD'], sched['n_lo_chunks'],
           tuple(sched['chunk_blk'].tolist()))
    if key not in _BUILD_CACHE:
        _BUILD_CACHE[key] = build_kernel(sched)
    nc = _BUILD_CACHE[key]

    Wp, Bp = pack_weights(P)
    EPAD = sched['EPAD']

    waug = np.zeros((4, 2 * 128), np.float32)
    waug[:3, 0:128] = P['enc_edge'][0][0]
    waug[3, 0:128] = P['enc_edge'][0][1]
    waug[:3, 128:256] = P['enc_node'][0][0]
    waug[3, 128:256] = P['enc_node'][0][1]
    waug = waug.astype(ml_dtypes.bfloat16)
    w4m2 = P['blocks'][1]['edge'][3][0].astype(ml_dtypes.bfloat16)
    iota = np.tile(np.arange(128, dtype=np.float32)[None, :], (128, 1))
    ibf = np.eye(128, dtype=np.float32).astype(ml_dtypes.bfloat16)
    if32 = np.eye(128, dtype=np.float32)

    in_maps = []
    for c in range(NC):
        cd = cores[c]
        ea = np.zeros((4, EPAD), np.float32)
        m = cd['eidx'] >= 0
        ea[:3, m] = edge_attr[cd['eidx'][m]].T
        ea[3, :] = 1.0
        ea = ea.astype(ml_dtypes.bfloat16)
        xa = np.zeros((4, NPAD), np.float32)
        xa[:3, :NPC] = x[c * NPC:(c + 1) * NPC].T
        xa[3, :] = 1.0
        xa = xa.astype(ml_dtypes.bfloat16)

        srow = cd['srow']
        s_local = np.where(srow >= THALF, srow - THALF, srow)
        sidx = wrap_idx16(s_local)
        rloc_v = np.where(cd['rloc'] >= 0, cd['rloc'], 0)
        ridx = wrap_idx16(rloc_v)
        rcvc = np.where(cd['rloc'] >= 0, cd['rloc'] % 128, -1).astype(
            np.float32).reshape(-1, 128).T.copy()
        deg = np.zeros(NPAD, np.float32)
        np.add.at(deg, cd['rloc'][cd['rloc'] >= 0], 1.0)
        degc = deg.reshape(NBLK, 128).T.copy()

        in_maps.append({
            "ea": ea, "x": xa, "w": Wp, "waug": waug, "w4m2bf": w4m2,
            "b": Bp, "iota": iota, "ibf": ibf, "if32": if32,
            "sidx": sidx, "ridx": ridx, "rcvc": rcvc, "deg": degc,
        })

    res = run_bass_kernel_spmd(nc, in_maps, core_ids=list(range(NC)))
    global DEBUG_OUT
    DEBUG_OUT = res.results
    out = np.zeros((N, 2), np.float32)
    for c in range(NC):
        out[c * NPC:(c + 1) * NPC] = res.results[c]["y"][:NPC]
    return out


DEBUG_OUT = None
